# revision 4
# baseline (speedup 1.0000x reference)
"""DigitCaps routing kernel for 8 Trainium2 NeuronCores.

Strategy: shard the input-capsule axis I=1152 as 144 per core (9 SBUF tiles of
128 = 16 i x 8 j). Per routing iteration each core computes its partial
s[b,o,d] = sum_{i local,j} exp(b_ij)[i,o] * W[i,o,d,j] * x[b,i,j] with K=128
matmuls, then one AllReduce fuses the s-partials with the softmax-denominator
partials (row 512 of the payload). The softmax normalization is folded into
squash. The agreement update is computed from G = sum_b x (x) v (PSUM
accumulation over the full batch), one elementwise multiply with W, a free-dim
reduction over d, and an indicator matmul that reduces over j on the PE.
"""
import numpy as np

import concourse.bacc as bacc
import concourse.mybir as mybir
import concourse.tile as tile
from concourse.bass_utils import run_bass_kernel_spmd

N_CORES = 8
B, I, O, D, J = 512, 1152, 10, 16, 8
IL = I // N_CORES          # 144 local input capsules
G = IL * J // 128          # 9 ij tiles of 128 partitions
M = B // 128               # 4 batch chunks
OD = O * D                 # 160
GO = G * O                 # 90
NIT = 3
F32 = mybir.dt.float32
Act = mybir.ActivationFunctionType
Alu = mybir.AluOpType

_cache = {}


def _build():
    nc = bacc.Bacc("TRN2", target_bir_lowering=False, debug=False, num_devices=N_CORES)
    xT_e = nc.dram_tensor("xT", [IL * J, B], F32, kind="ExternalInput")
    xN_e = nc.dram_tensor("xN", [B, IL * J], F32, kind="ExternalInput")
    w2_e = nc.dram_tensor("w2", [IL * J, OD], F32, kind="ExternalInput")
    ind_e = nc.dram_tensor("ind", [16, 128], F32, kind="ExternalInput")
    indj_e = nc.dram_tensor("indj", [128, 16], F32, kind="ExternalInput")
    v_e = nc.dram_tensor("v_out", [B, OD], F32, kind="ExternalOutput")

    with tile.TileContext(nc) as tc:
        with (
            tc.tile_pool(name="const", bufs=1) as constp,
            tc.tile_pool(name="big", bufs=1) as big,
            tc.tile_pool(name="work", bufs=2) as work,
            tc.tile_pool(name="ps_s", bufs=2, space="PSUM") as ps_s_pool,
            tc.tile_pool(name="ps_g", bufs=2, space="PSUM") as ps_g_pool,
            tc.tile_pool(name="ps_c", bufs=2, space="PSUM") as ps_c_pool,
            tc.tile_pool(name="ps_m", bufs=1, space="PSUM") as ps_m_pool,
            tc.tile_pool(name="dram", bufs=2, space="DRAM") as dram,
        ):
            # ---- persistent inputs ----
            xT = big.tile([128, G * B], F32)        # [p=(i16,j8), (g, b)]
            for g in range(G):
                nc.sync.dma_start(out=xT[:, g * B:(g + 1) * B],
                                  in_=xT_e[g * 128:(g + 1) * 128, :])
            xN = big.tile([128, M * IL * J], F32)   # [p=b, (m, ij)]
            for m in range(M):
                nc.sync.dma_start(out=xN[:, m * IL * J:(m + 1) * IL * J],
                                  in_=xN_e[m * 128:(m + 1) * 128, :])
            w2 = big.tile([128, G * OD], F32)       # [p=(i16,j8), (g, o, d)]
            for g in range(G):
                nc.sync.dma_start(out=w2[:, g * OD:(g + 1) * OD],
                                  in_=w2_e[g * 128:(g + 1) * 128, :])
            ind = constp.tile([16, 128], F32)
            nc.sync.dma_start(out=ind[:], in_=ind_e[:])
            indj = constp.tile([128, 16], F32)
            nc.sync.dma_start(out=indj[:], in_=indj_e[:])
            ones1 = constp.tile([1, 128], F32)
            nc.vector.memset(ones1[:], 1.0)
            ones16 = constp.tile([16, 1], F32)
            nc.vector.memset(ones16[:], 1.0)

            denrow = constp.tile([1, OD], F32)      # AR payload row 512
            nc.vector.memset(denrow[:], 0.0)
            # iteration 1: b=0 -> e=1 -> per-core denominator partial = IL
            nc.vector.memset(denrow[0:1, 0:O], float(IL))

            b_tiles = [big.tile([16, GO], F32, name=f"bstate{i}") for i in range(NIT - 1)]

            for t in range(NIT):
                last = t == NIT - 1
                if t > 0:
                    # e = exp(b); local denominator partial
                    e_sb = work.tile([16, GO], F32)
                    nc.scalar.activation(e_sb[:], b_tiles[t - 1][:], Act.Exp)
                    ps_misc = ps_m_pool.tile([128, 512], F32, name="ps_misc", tag="misc")
                    nc.tensor.matmul(ps_misc[0:1, 0:GO], ones16[:], e_sb[:],
                                     start=True, stop=True)
                    dview = ps_misc[0:1, 0:GO].rearrange("p (g o) -> p o g", g=G)
                    nc.vector.reduce_sum(denrow[0:1, 0:O], dview,
                                         axis=mybir.AxisListType.X)
                    # Wc = w2 * broadcast(e) per tile g
                    wc = work.tile([128, G * OD], F32)
                    for g in range(G):
                        ps_ce = ps_c_pool.tile([128, OD], F32)
                        rhs = e_sb[:, g * O:(g + 1) * O].unsqueeze(2) \
                            .broadcast_to([16, O, D])
                        nc.tensor.matmul(ps_ce[:], ind[:], rhs, start=True, stop=True)
                        nc.vector.tensor_tensor(
                            wc[:, g * OD:(g + 1) * OD],
                            w2[:, g * OD:(g + 1) * OD], ps_ce[:], op=Alu.mult)
                    s_rhs = wc
                else:
                    s_rhs = w2

                # partial s: [b, od] accumulated over the 9 local ij tiles
                ar_in = dram.tile([B + 1, OD], F32)
                ar_out = dram.tile([B + 1, OD], F32)
                s_stage = work.tile([128, M * OD], F32)
                for m in range(M):
                    ps_s = ps_s_pool.tile([128, OD], F32)
                    for g in range(G):
                        nc.tensor.matmul(
                            ps_s[:],
                            xT[:, g * B + m * 128: g * B + (m + 1) * 128],
                            s_rhs[:, g * OD:(g + 1) * OD],
                            start=(g == 0), stop=(g == G - 1))
                    nc.scalar.activation(s_stage[:, m * OD:(m + 1) * OD], ps_s[:],
                                         Act.Copy)
                    nc.sync.dma_start(out=ar_in[m * 128:(m + 1) * 128, :],
                                      in_=s_stage[:, m * OD:(m + 1) * OD])
                nc.sync.dma_start(out=ar_in[B:B + 1, :], in_=denrow[:])
                nc.gpsimd.collective_compute(
                    "AllReduce", Alu.add,
                    replica_groups=[list(range(N_CORES))],
                    ins=[ar_in.opt()], outs=[ar_out.opt()])

                # global s + denominator back to SBUF
                s_sb = work.tile([128, M * OD], F32)
                for m in range(M):
                    nc.sync.dma_start(out=s_sb[:, m * OD:(m + 1) * OD],
                                      in_=ar_out[m * 128:(m + 1) * 128, :])
                deng = work.tile([1, O], F32)
                nc.sync.dma_start(out=deng[:], in_=ar_out[B:B + 1, 0:O])
                invd = work.tile([1, O], F32)
                nc.vector.reciprocal(invd[:], deng[:])
                invd2 = work.tile([1, O], F32)
                nc.vector.tensor_tensor(invd2[:], invd[:], invd[:], op=Alu.mult)
                ps_misc2 = ps_m_pool.tile([128, 512], F32, name="ps_misc2", tag="misc")
                nc.tensor.matmul(ps_misc2[:, 0:O], ones1[:], invd[:],
                                 start=True, stop=True)
                nc.tensor.matmul(ps_misc2[:, 16:16 + O], ones1[:], invd2[:],
                                 start=True, stop=True)

                # squash with folded normalization:
                # v = s_raw * invd * sqrt(sq)/(1+sq),  sq = invd^2 * sum_d s_raw^2
                sqr = work.tile([128, M * OD], F32)
                nc.scalar.activation(sqr[:], s_sb[:], Act.Square)
                sqs = work.tile([128, M * O], F32)
                nc.vector.reduce_sum(
                    sqs[:], sqr[:].rearrange("p (m o d) -> p m o d", m=M, o=O),
                    axis=mybir.AxisListType.X)
                sqt = work.tile([128, M * O], F32)
                nc.vector.tensor_tensor(
                    sqt[:].rearrange("p (m o) -> p m o", m=M),
                    sqs[:].rearrange("p (m o) -> p m o", m=M),
                    ps_misc2[:, 16:16 + O].unsqueeze(1).broadcast_to([128, M, O]),
                    op=Alu.mult)
                rt = work.tile([128, M * O], F32)
                nc.scalar.activation(rt[:], sqt[:], Act.Sqrt)
                d2 = work.tile([128, M * O], F32)
                nc.vector.tensor_scalar_add(d2[:], sqt[:], 1.0)
                rc = work.tile([128, M * O], F32)
                nc.vector.reciprocal(rc[:], d2[:])
                gf = work.tile([128, M * O], F32)
                nc.vector.tensor_tensor(gf[:], rt[:], rc[:], op=Alu.mult)
                gf2 = work.tile([128, M * O], F32)
                nc.vector.tensor_tensor(
                    gf2[:].rearrange("p (m o) -> p m o", m=M),
                    gf[:].rearrange("p (m o) -> p m o", m=M),
                    ps_misc2[:, 0:O].unsqueeze(1).broadcast_to([128, M, O]),
                    op=Alu.mult)
                v_sb = work.tile([128, M * OD], F32)
                nc.vector.tensor_tensor(
                    v_sb[:].rearrange("p (m o d) -> p m o d", m=M, o=O),
                    s_sb[:].rearrange("p (m o d) -> p m o d", m=M, o=O),
                    gf2[:].rearrange("p (m o) -> p m o", m=M).unsqueeze(3)
                    .broadcast_to([128, M, O, D]),
                    op=Alu.mult)

                if last:
                    for m in range(M):
                        nc.sync.dma_start(out=v_e[m * 128:(m + 1) * 128, :],
                                          in_=v_sb[:, m * OD:(m + 1) * OD])
                    continue

                # G4[g] = sum_b x[b, ij] v[b, od]; P4 = w2 * G4; reduce d; sum j
                p4d = work.tile([128, GO], F32)
                p4 = work.tile([128, G * OD], F32)
                for g in range(G):
                    ps_g = ps_g_pool.tile([128, OD], F32)
                    for m in range(M):
                        nc.tensor.matmul(
                            ps_g[:],
                            xN[:, m * IL * J + g * 128: m * IL * J + (g + 1) * 128],
                            v_sb[:, m * OD:(m + 1) * OD],
                            start=(m == 0), stop=(m == M - 1))
                    nc.vector.tensor_tensor(
                        p4[:, g * OD:(g + 1) * OD],
                        w2[:, g * OD:(g + 1) * OD], ps_g[:], op=Alu.mult)
                nc.vector.reduce_sum(
                    p4d[:], p4[:].rearrange("p (go d) -> p go d", d=D),
                    axis=mybir.AxisListType.X)
                ps_b = ps_m_pool.tile([16, GO], F32)
                for g in range(G):
                    nc.tensor.matmul(ps_b[:, g * O:(g + 1) * O], indj[:],
                                     p4d[:, g * O:(g + 1) * O],
                                     start=True, stop=True)
                if t == 0:
                    nc.vector.tensor_copy(b_tiles[0][:], ps_b[:])
                else:
                    nc.vector.tensor_tensor(b_tiles[t][:], b_tiles[t - 1][:],
                                            ps_b[:], op=Alu.add)

    nc.compile()
    return nc


def _host_inputs(x, W):
    """Slice + lay out per-core inputs."""
    x = np.ascontiguousarray(x, dtype=np.float32)
    W = np.ascontiguousarray(W, dtype=np.float32)
    ind = np.zeros((16, 128), dtype=np.float32)
    for k in range(16):
        ind[k, k * 8:(k + 1) * 8] = 1.0
    indj = np.ascontiguousarray(ind.T) / float(B)
    in_maps = []
    for c in range(N_CORES):
        sl = slice(c * IL, (c + 1) * IL)
        xs = x[:, sl, :].reshape(B, IL * J)
        ws = W[sl]  # [IL, O, D, J]
        in_maps.append({
            "xT": np.ascontiguousarray(xs.T),
            "xN": np.ascontiguousarray(xs),
            "w2": np.ascontiguousarray(
                ws.transpose(0, 3, 1, 2).reshape(IL * J, OD)),
            "ind": ind,
            "indj": indj,
        })
    return in_maps


def kernel(x, W):
    if "nc" not in _cache:
        _cache["nc"] = _build()
    nc = _cache["nc"]
    in_maps = _host_inputs(x, W)
    res = run_bass_kernel_spmd(nc, in_maps, list(range(N_CORES)))
    v = res.results[0]["v_out"]
    return v.reshape(B, O, D, 1).astype(np.float32)


# revision 5
# speedup vs baseline: 590.6418x; 590.6418x over previous
"""DigitCaps routing kernel for 8 Trainium2 NeuronCores.

Strategy: shard the input-capsule axis I=1152 as 144 per core (9 SBUF tiles of
128 = 16 i x 8 j). Per routing iteration each core computes its partial
s[b,o,d] = sum_{i local,j} exp(b_ij)[i,o] * W[i,o,d,j] * x[b,i,j] with K=128
matmuls, then one AllReduce fuses the s-partials with the softmax-denominator
partials (row 512 of the payload). The softmax normalization is folded into
squash. The agreement update is computed from G = sum_b x (x) v (PSUM
accumulation over the full batch), one elementwise multiply with W, a free-dim
reduction over d, and an indicator matmul that reduces over j on the PE.

All small matmuls are batched into wide-N matmuls (N=480) to minimize
PE->PSUM->DVE handoffs, which dominate the serial chain cost on this part.
"""
import numpy as np

import concourse.bacc as bacc
import concourse.mybir as mybir
import concourse.tile as tile
from concourse.bass_utils import run_bass_kernel_spmd

N_CORES = 8
B, I, O, D, J = 512, 1152, 10, 16, 8
IL = I // N_CORES          # 144 local input capsules
G = IL * J // 128          # 9 ij tiles of 128 partitions
M = B // 128               # 4 batch chunks
C = 3                      # g-chunks of 3 tiles -> N=480 wide ops
OD = O * D                 # 160
GO = G * O                 # 90
W3 = 3 * OD                # 480
NIT = 3
F32 = mybir.dt.float32
Act = mybir.ActivationFunctionType
Alu = mybir.AluOpType

_cache = {}


def _build(repeat=1, no_ar=False, skip_wc=False, skip_agree=False):
    nc = bacc.Bacc("TRN2", target_bir_lowering=False, debug=False, num_devices=N_CORES)
    xT_e = nc.dram_tensor("xT", [IL * J, B], F32, kind="ExternalInput")
    xN_e = nc.dram_tensor("xN", [B, IL * J], F32, kind="ExternalInput")
    w2_e = nc.dram_tensor("w2", [IL * J, OD], F32, kind="ExternalInput")
    ind_e = nc.dram_tensor("ind", [16, 128], F32, kind="ExternalInput")
    indj_e = nc.dram_tensor("indj", [128, 16], F32, kind="ExternalInput")
    v_e = nc.dram_tensor("v_out", [B, OD], F32, kind="ExternalOutput")

    with tile.TileContext(nc) as tc:
        with (
            tc.tile_pool(name="const", bufs=1) as constp,
            tc.tile_pool(name="big", bufs=1) as big,
            tc.tile_pool(name="work", bufs=2) as work,
            tc.tile_pool(name="ps_s", bufs=2, space="PSUM") as ps_s_pool,
            tc.tile_pool(name="ps_g", bufs=2, space="PSUM") as ps_g_pool,
            tc.tile_pool(name="ps_c", bufs=2, space="PSUM") as ps_c_pool,
            tc.tile_pool(name="ps_m", bufs=1, space="PSUM") as ps_m_pool,
            tc.tile_pool(name="dram", bufs=2, space="DRAM") as dram,
        ):
            # ---- persistent inputs ----
            xT = big.tile([128, G * B], F32)        # [p=(i16,j8), (g, b)]
            for g in range(G):
                nc.sync.dma_start(out=xT[:, g * B:(g + 1) * B],
                                  in_=xT_e[g * 128:(g + 1) * 128, :])
            xN = big.tile([128, M * IL * J], F32)   # [p=b, (m, ij)]
            for m in range(M):
                nc.sync.dma_start(out=xN[:, m * IL * J:(m + 1) * IL * J],
                                  in_=xN_e[m * 128:(m + 1) * 128, :])
            w2 = big.tile([128, G * OD], F32)       # [p=(i16,j8), (g, o, d)]
            for g in range(G):
                nc.sync.dma_start(out=w2[:, g * OD:(g + 1) * OD],
                                  in_=w2_e[g * 128:(g + 1) * 128, :])
            ind = constp.tile([16, 128], F32)
            nc.sync.dma_start(out=ind[:], in_=ind_e[:])
            indj = constp.tile([128, 16], F32)
            nc.sync.dma_start(out=indj[:], in_=indj_e[:])
            ones1 = constp.tile([1, 128], F32)
            nc.vector.memset(ones1[:], 1.0)
            ones16 = constp.tile([16, 1], F32)
            nc.vector.memset(ones16[:], 1.0)

            denrow = constp.tile([1, OD], F32)      # AR payload row 512
            nc.vector.memset(denrow[:], 0.0)
            # iteration 1: b=0 -> e=1 -> per-core denominator partial = IL
            nc.vector.memset(denrow[0:1, 0:O], float(IL))

            for rep in range(repeat):
              b_tiles = [big.tile([16, GO], F32, name=f"bstate{rep}_{i}")
                         for i in range(NIT - 1)]
              for t in range(NIT):
                last = t == NIT - 1
                if t > 0 and not skip_agree:
                    # e = exp(b); local denominator partial
                    e_sb = work.tile([16, GO], F32)
                    nc.scalar.activation(e_sb[:], b_tiles[t - 1][:], Act.Exp)
                    ps_misc = ps_m_pool.tile([128, 512], F32,
                                             name="ps_misc", tag="misc")
                    nc.tensor.matmul(ps_misc[0:1, 96:96 + GO], ones16[:], e_sb[:],
                                     start=True, stop=True)
                    dview = ps_misc[0:1, 96:96 + GO].rearrange(
                        "p (g o) -> p o g", g=G)
                    nc.vector.reduce_sum(denrow[0:1, 0:O], dview,
                                         axis=mybir.AxisListType.X)
                    # Wc = w2 * broadcast(e): 3 wide MMs (N=480) + 3 wide TTs
                    wc = work.tile([128, G * OD], F32)
                    for c in range(C):
                        ps_ce = ps_c_pool.tile([128, W3], F32)
                        rhs = e_sb[:, c * 30:(c + 1) * 30] \
                            .rearrange("p (g o) -> p g o", g=3) \
                            .unsqueeze(3).broadcast_to([16, 3, O, D])
                        nc.tensor.matmul(ps_ce[:], ind[:], rhs,
                                         start=True, stop=True)
                        nc.vector.tensor_tensor(
                            wc[:, c * W3:(c + 1) * W3],
                            w2[:, c * W3:(c + 1) * W3], ps_ce[:], op=Alu.mult)
                    s_rhs = w2 if skip_wc else wc
                else:
                    s_rhs = w2

                # partial s: [b, od] accumulated over the 9 local ij tiles
                ar_in = dram.tile([B + 1, OD], F32)
                ar_out = dram.tile([B + 1, OD], F32)
                s_stage = work.tile([128, M * OD], F32)
                for m in range(M):
                    ps_s = ps_s_pool.tile([128, OD], F32)
                    for g in range(G):
                        nc.tensor.matmul(
                            ps_s[:],
                            xT[:, g * B + m * 128: g * B + (m + 1) * 128],
                            s_rhs[:, g * OD:(g + 1) * OD],
                            start=(g == 0), stop=(g == G - 1))
                    nc.scalar.activation(s_stage[:, m * OD:(m + 1) * OD], ps_s[:],
                                         Act.Copy)
                # one merged DMA: s_stage [p, (m, od)] -> ar rows (m*128+p)
                nc.sync.dma_start(
                    out=ar_in[0:B, :].rearrange("(m p) od -> p m od", m=M),
                    in_=s_stage[:].rearrange("p (m od) -> p m od", m=M))
                nc.sync.dma_start(out=ar_in[B:B + 1, :], in_=denrow[:])
                if no_ar:
                    nc.sync.dma_start(out=ar_out[:, :], in_=ar_in[:, :])
                else:
                    nc.gpsimd.collective_compute(
                        "AllReduce", Alu.add,
                        replica_groups=[list(range(N_CORES))],
                        ins=[ar_in.opt()], outs=[ar_out.opt()])

                # global s + denominator back to SBUF (one merged DMA)
                s_sb = work.tile([128, M * OD], F32)
                nc.sync.dma_start(
                    out=s_sb[:].rearrange("p (m od) -> p m od", m=M),
                    in_=ar_out[0:B, :].rearrange("(m p) od -> p m od", m=M))
                ivp = work.tile([1, 32], F32)
                nc.sync.dma_start(out=ivp[0:1, 0:O], in_=ar_out[B:B + 1, 0:O])
                nc.vector.reciprocal(ivp[0:1, 0:O], ivp[0:1, 0:O])
                nc.vector.tensor_tensor(ivp[0:1, 16:16 + O], ivp[0:1, 0:O],
                                        ivp[0:1, 0:O], op=Alu.mult)
                ps_bc = ps_m_pool.tile([128, 512], F32, name="ps_bc", tag="misc")
                nc.tensor.matmul(ps_bc[:, 0:32], ones1[:], ivp[:],
                                 start=True, stop=True)
                iv1 = ps_bc[:, 0:O]        # invden broadcast [128, 10]
                iv2 = ps_bc[:, 16:16 + O]  # invden^2 broadcast [128, 10]

                # squash with folded normalization:
                # v = s_raw * invd * sqrt(sq)/(1+sq),  sq = invd^2 * sum_d s_raw^2
                sqr = work.tile([128, M * OD], F32)
                nc.scalar.activation(sqr[:], s_sb[:], Act.Square)
                sqs = work.tile([128, M * O], F32)
                nc.vector.reduce_sum(
                    sqs[:], sqr[:].rearrange("p (m o d) -> p m o d", m=M, o=O),
                    axis=mybir.AxisListType.X)
                sqt = work.tile([128, M * O], F32)
                nc.vector.tensor_tensor(
                    sqt[:].rearrange("p (m o) -> p m o", m=M),
                    sqs[:].rearrange("p (m o) -> p m o", m=M),
                    iv2.unsqueeze(1).broadcast_to([128, M, O]), op=Alu.mult)
                rt = work.tile([128, M * O], F32)
                nc.scalar.activation(rt[:], sqt[:], Act.Sqrt)
                d2 = work.tile([128, M * O], F32)
                nc.vector.tensor_scalar_add(d2[:], sqt[:], 1.0)
                rc = work.tile([128, M * O], F32)
                nc.vector.reciprocal(rc[:], d2[:])
                gf = work.tile([128, M * O], F32)
                nc.vector.tensor_tensor(gf[:], rt[:], rc[:], op=Alu.mult)
                gf2 = work.tile([128, M * O], F32)
                nc.vector.tensor_tensor(
                    gf2[:].rearrange("p (m o) -> p m o", m=M),
                    gf[:].rearrange("p (m o) -> p m o", m=M),
                    iv1.unsqueeze(1).broadcast_to([128, M, O]), op=Alu.mult)
                v_sb = work.tile([128, M * OD], F32)
                nc.vector.tensor_tensor(
                    v_sb[:].rearrange("p (m o d) -> p m o d", m=M, o=O),
                    s_sb[:].rearrange("p (m o d) -> p m o d", m=M, o=O),
                    gf2[:].rearrange("p (m o) -> p m o", m=M).unsqueeze(3)
                    .broadcast_to([128, M, O, D]),
                    op=Alu.mult)

                if last:
                    nc.sync.dma_start(
                        out=v_e[:, :].rearrange("(m p) od -> p m od", m=M),
                        in_=v_sb[:].rearrange("p (m od) -> p m od", m=M))
                    continue
                if skip_agree:
                    continue

                # G4 = sum_b x (x) v; P4 = w2*G4; reduce d; reduce j on PE
                p4 = work.tile([128, G * OD], F32)
                for c in range(C):
                    ps_g = ps_g_pool.tile([128, W3], F32)
                    for gg in range(3):
                        g = c * 3 + gg
                        for m in range(M):
                            nc.tensor.matmul(
                                ps_g[:, gg * OD:(gg + 1) * OD],
                                xN[:, m * IL * J + g * 128: m * IL * J + (g + 1) * 128],
                                v_sb[:, m * OD:(m + 1) * OD],
                                start=(m == 0), stop=(m == M - 1))
                    nc.vector.tensor_tensor(
                        p4[:, c * W3:(c + 1) * W3],
                        w2[:, c * W3:(c + 1) * W3], ps_g[:], op=Alu.mult)
                p4d = work.tile([128, GO], F32)
                nc.vector.reduce_sum(
                    p4d[:], p4[:].rearrange("p (go d) -> p go d", d=D),
                    axis=mybir.AxisListType.X)
                ps_b = ps_m_pool.tile([16, GO], F32, name="ps_b", tag="psb")
                for c in range(C):
                    nc.tensor.matmul(ps_b[:, c * 30:(c + 1) * 30], indj[:],
                                     p4d[:, c * 30:(c + 1) * 30],
                                     start=True, stop=True)
                if t == 0:
                    nc.vector.tensor_copy(b_tiles[0][:], ps_b[:])
                else:
                    nc.vector.tensor_tensor(b_tiles[t][:], b_tiles[t - 1][:],
                                            ps_b[:], op=Alu.add)

    nc.compile()
    return nc


def _host_inputs(x, W):
    """Slice + lay out per-core inputs."""
    x = np.ascontiguousarray(x, dtype=np.float32)
    W = np.ascontiguousarray(W, dtype=np.float32)
    ind = np.zeros((16, 128), dtype=np.float32)
    for k in range(16):
        ind[k, k * 8:(k + 1) * 8] = 1.0
    indj = np.ascontiguousarray(ind.T) / float(B)
    in_maps = []
    for c in range(N_CORES):
        sl = slice(c * IL, (c + 1) * IL)
        xs = x[:, sl, :].reshape(B, IL * J)
        ws = W[sl]  # [IL, O, D, J]
        in_maps.append({
            "xT": np.ascontiguousarray(xs.T),
            "xN": np.ascontiguousarray(xs),
            "w2": np.ascontiguousarray(
                ws.transpose(0, 3, 1, 2).reshape(IL * J, OD)),
            "ind": ind,
            "indj": indj,
        })
    return in_maps


def kernel(x, W):
    if "nc" not in _cache:
        _cache["nc"] = _build()
    nc = _cache["nc"]
    in_maps = _host_inputs(x, W)
    res = run_bass_kernel_spmd(nc, in_maps, list(range(N_CORES)))
    v = res.results[0]["v_out"]
    return v.reshape(B, O, D, 1).astype(np.float32)
